# revision 13
# baseline (speedup 1.0000x reference)
"""Trainium2 Bass kernel for nn_Cross_attention_2 (sparse_attention).

Math (B=1, C=32, D=36, H=W=48, P=9):
  xc = conv1x1(x, W_img, b_img)            # per-voxel channel mix
  v  = unfold(xc)                          # (C, L=1024, 81) non-overlapping 9x9 patches
  px = LeakyReLU(v @ (W2@W1)^T + bias)     # the two Linears collapse to A = W2@W1
  att[c] = px[c] @ py[c]^T / 81            # (C, 1024, 1024)

Sharding: channels C=32 split across 8 cores (4 each). Params replicated
(per-core slices precomputed on host). Each core reads full x, y.

v2: bf16 end-to-end I/O (inputs staged bf16, att written bf16 and upcast on
host; rel tol 2e-2 leaves ample margin), fp32 PSUM accumulation throughout.

Per-core device pipeline:
  conv:      3 accumulating zero-padded block-diag matmuls (K=128/128/32)
             -> xc (37, 4, 2304) bf16: rows kd*4+o, row 36 = 1.0 (bias row)
  transform: unfold folded into strided rhs APs; 9 kw-accumulation passes per
             output tile (k=37); combined weight TM = channel select + A +
             bias. LeakyReLU epilogue = scalar mul 0.2 + vector max.
  att:       pxT/pyT kept as (81, 1024) bf16; out tiles (128, 512) fp32 PSUM,
             cast to bf16 and batched into 1 MiB output DMAs (4 row-tiles).
             transform+att interleaved per channel so output DMA starts early.
"""

import sys

sys.path.insert(0, "/opt/trn_rl_repo")

import contextlib
import os

import numpy as np

import concourse.bass as bass  # noqa: F401
import concourse.tile as tile
from concourse import bacc, mybir
from concourse.bass_utils import run_bass_kernel_spmd

P = 9
P2 = 81
C = 32
D = 36
HWF = 2304
ND = 4  # pd blocks (D/9)
L = 1024
N_CORES = 8
CPC = 4  # channels per core


F32 = mybir.dt.float32
BF16 = mybir.dt.bfloat16
BF16_NP = mybir.dt.np(mybir.dt.bfloat16)

_CACHE = {}
last_results = None  # BassKernelResults of the most recent run (for test.py)

_HW_CHUNKS = [(0, 512), (512, 512), (1024, 512), (1536, 512), (2048, 256)]
_KD_PASSES = [(0, 4), (4, 4), (8, 1)]  # (kd0, nkd) conv passes


def _build():
    if "nc" in _CACHE:
        return _CACHE["nc"]

    nc = bacc.Bacc("TRN2", target_bir_lowering=False, debug=False,
                   num_devices=N_CORES)
    x_d = nc.dram_tensor("x", (C, D, HWF), BF16, kind="ExternalInput").ap()
    y_d = nc.dram_tensor("y", (C, D, HWF), BF16, kind="ExternalInput").ap()
    # wblk: (128, 216) = conv lhsT for (t in 2) x (pass i in 3), 36 cols each
    wblk_d = nc.dram_tensor("wblk", (128, 216), BF16, kind="ExternalInput").ap()
    # tm: (37, 2*4*9*81) combined transform weights in SBUF layout
    tm_d = nc.dram_tensor("tm", (37, 2 * CPC * P * P2), BF16,
                          kind="ExternalInput").ap()
    ones_d = nc.dram_tensor("ones", (1, ND * HWF), BF16,
                            kind="ExternalInput").ap()
    # att rows split (mb, m, p): l1 = mb*512 + m*128 + p
    att_d = nc.dram_tensor("att", (CPC, 2, 4, 128, L), BF16,
                           kind="ExternalOutput").ap()

    with tile.TileContext(nc) as tc:
        with contextlib.ExitStack() as ctx:
            consts = ctx.enter_context(tc.tile_pool(name="consts", bufs=1))
            xbp = ctx.enter_context(tc.tile_pool(name="xb", bufs=3))
            xbp2 = ctx.enter_context(tc.tile_pool(name="xb2", bufs=1))
            tmpp = ctx.enter_context(tc.tile_pool(name="tmp", bufs=2))
            outp = ctx.enter_context(tc.tile_pool(name="outp", bufs=2))
            cps = ctx.enter_context(tc.tile_pool(name="cps", bufs=2, space="PSUM"))
            tps = ctx.enter_context(tc.tile_pool(name="tps", bufs=2, space="PSUM"))
            aps = ctx.enter_context(tc.tile_pool(name="aps", bufs=3, space="PSUM"))

            wb_sb = consts.tile([128, 216], BF16, tag="wb")
            nc.sync.dma_start(out=wb_sb[:, :], in_=wblk_d[:, :])
            tm_sb = consts.tile([37, 2 * CPC * P * P2], BF16, tag="tm")
            nc.sync.dma_start(out=tm_sb[:, :], in_=tm_d[:, :])
            tm_v = tm_sb.rearrange("p (t c kw j) -> p t c kw j", t=2, c=CPC,
                                   kw=P)

            xc_sb = []
            px_sb = []
            for t in range(2):
                xt = consts.tile([37, ND, HWF], BF16, tag=f"xc{t}")
                nc.sync.dma_start(
                    out=xt[36:37, :, :],
                    in_=ones_d.rearrange("p (d h) -> p d h", d=ND))
                xc_sb.append(xt)
                px_sb.append([consts.tile([P2, L], BF16, tag=f"px{t}{c}",
                                          name=f"px{t}{c}")
                              for c in range(CPC)])

            for t in range(2):
                src = x_d if t == 0 else y_d
                for pd in range(ND):
                    xbs = []
                    for i, (kd0, nkd) in enumerate(_KD_PASSES):
                        kp = 32 * nkd
                        pool = xbp if nkd == 4 else xbp2
                        xb = pool.tile([kp, HWF], BF16, tag=f"xb{min(i, 1)}",
                                       name=f"xb{min(i, 1)}")
                        rows = src[:, pd * P + kd0: pd * P + kd0 + nkd, :]
                        nc.sync.dma_start(out=xb[:, :],
                                          in_=rows.transpose([1, 0, 2]))
                        xbs.append(xb)
                    for h0, hn in _HW_CHUNKS:
                        ps = cps.tile([36, 512], F32, tag="cps")
                        for i, (kd0, nkd) in enumerate(_KD_PASSES):
                            kp = 32 * nkd
                            lhs = wb_sb[0:kp,
                                        (t * 3 + i) * 36: (t * 3 + i + 1) * 36]
                            nc.tensor.matmul(
                                ps[:, :hn], lhs, xbs[i][:, h0: h0 + hn],
                                start=(i == 0), stop=(i == 2))
                        dst = xc_sb[t][0:36, pd, h0: h0 + hn]
                        if (h0 // 512) % 2 == 0:
                            nc.vector.tensor_copy(out=dst, in_=ps[:, :hn])
                        else:
                            nc.scalar.copy(out=dst, in_=ps[:, :hn])


            # transform + att interleaved per channel so output DMA starts early
            for c in range(CPC):
                for t in range(2):
                    for ch in range(2):  # l-chunks of 512 (pd pairs)
                        zp = tps.tile([P2, 512], F32, tag="tps")
                        for kw in range(P):
                            rhs = xc_sb[t][:, 2 * ch: 2 * ch + 2, kw:HWF:P]
                            nc.tensor.matmul(
                                zp[:, :], tm_v[:, t, c, kw, :], rhs,
                                start=(kw == 0), stop=(kw == P - 1))
                        # LeakyReLU(z) = max(0.2*z, z), cast to bf16
                        zm = tmpp.tile([P2, 512], F32, tag="zm")
                        nc.scalar.mul(zm[:, :], zp[:, :], 0.2)
                        nc.vector.tensor_tensor(
                            out=px_sb[t][c][:, ch * 512: ch * 512 + 512],
                            in0=zp[:, :], in1=zm[:, :],
                            op=mybir.AluOpType.max)

                # att[c] = pxT[c].T @ pyT[c]
                for mb in range(2):
                    ob = outp.tile([128, 4, L], BF16, tag="ob")
                    for mi in range(4):
                        m = mb * 4 + mi
                        for nch in range(2):
                            ap_ = aps.tile([128, 512], F32, tag="aps")
                            nc.tensor.matmul(
                                ap_[:, :],
                                px_sb[0][c][:, m * 128: m * 128 + 128],
                                px_sb[1][c][:, nch * 512: nch * 512 + 512],
                                start=True, stop=True)
                            dst = ob[:, mi, nch * 512: nch * 512 + 512]
                            if (mi * 2 + nch) % 2 == 0:
                                nc.vector.tensor_copy(out=dst, in_=ap_[:, :])
                            else:
                                nc.scalar.copy(out=dst, in_=ap_[:, :])
                    nc.sync.dma_start(
                        out=att_d[c, mb].transpose([1, 0, 2]),
                        in_=ob[:, :, :])

    nc.compile()
    _CACHE["nc"] = nc
    return nc


def _host_prep(x, y, W_img, b_img, W_fea, b_fea, W1, W2):
    """Build per-core wblk / tm arrays. Returns in_maps list."""
    x = np.ascontiguousarray(
        np.asarray(x, np.float32).reshape(C, D, HWF)).astype(BF16_NP)
    y = np.ascontiguousarray(
        np.asarray(y, np.float32).reshape(C, D, HWF)).astype(BF16_NP)
    W_img = np.asarray(W_img, np.float32)
    b_img = np.asarray(b_img, np.float32)
    W_fea = np.asarray(W_fea, np.float32)
    b_fea = np.asarray(b_fea, np.float32)
    A = np.asarray(W2, np.float32) @ np.asarray(W1, np.float32)  # (81, 81)
    rowsum = A.sum(axis=1)  # (81,)
    At = np.stack([A / P2, A])                 # x-side carries the 1/81
    ones = np.ones((1, ND * HWF), BF16_NP)

    in_maps = []
    for r in range(N_CORES):
        Wl = [W_img[r * CPC:(r + 1) * CPC, :], W_fea[r * CPC:(r + 1) * CPC, :]]
        bl = [b_img[r * CPC:(r + 1) * CPC], b_fea[r * CPC:(r + 1) * CPC]]

        # conv lhsT: wblk[kd_l*32+c', (t*3+i)*36 + kd*4+o] = W_t[o, c']
        #            with kd = kd0_i + kd_l
        wblk = np.zeros((128, 216), np.float32)
        for t in range(2):
            for i, (kd0, nkd) in enumerate(_KD_PASSES):
                for kd_l in range(nkd):
                    kd = kd0 + kd_l
                    rows = slice(kd_l * 32, kd_l * 32 + 32)
                    for o in range(CPC):
                        col = (t * 3 + i) * 36 + kd * 4 + o
                        wblk[rows, col] = Wl[t][o, :]

        # tm[p, t, c, kw, j]; p = kd*4 + o, row 36 = bias (kw=0 only)
        tm = np.zeros((37, 2, CPC, P, P2), np.float32)
        bias = np.stack([np.outer(bl[0], rowsum) / P2,
                         np.outer(bl[1], rowsum)])  # (2, 4, 81)
        for kd in range(P):
            for o in range(CPC):
                p = kd * 4 + o
                # tm[p, t, o, kw, j] = At[t, j, kd*9+kw]
                tm[p, :, o, :, :] = At[:, :, kd * P:(kd + 1) * P].transpose(0, 2, 1)
        tm[36, :, :, 0, :] = bias
        tm = tm.reshape(37, 2 * CPC * P * P2)

        in_maps.append({"x": x, "y": y,
                        "wblk": wblk.astype(BF16_NP),
                        "tm": np.ascontiguousarray(tm).astype(BF16_NP),
                        "ones": ones})
    return in_maps


def kernel(**inputs):
    global last_results
    nc = _build()
    in_maps = _host_prep(**inputs)
    trace = bool(os.environ.get("KERNEL_TRACE"))
    res = run_bass_kernel_spmd(nc, in_maps, core_ids=list(range(N_CORES)),
                               trace=trace)
    last_results = res
    att = np.stack([res.results[r]["att"].astype(np.float32)
                    for r in range(N_CORES)])
    return att.reshape(1, C, L, L)
